# revision 62
# baseline (speedup 1.0000x reference)
"""Causal MHA (B=4, T=2048, D=1024, H=16, Dh=64) on 8 TRN2 NeuronCores.

Sharding: tensor-parallel over heads (2 groups of 8 heads; W_q/W_k/W_v split
column-wise, W_o row-wise) x data-parallel over batch (4 batches). Core
c = (b, g) computes a partial output x[b] attention with head-group g; the
host sums the two head-group partials per batch.

All device matmuls run in bf16 (fp32 PSUM accumulation); verified numerically
at ~4e-3 rel err vs the fp32 reference (tolerance 2e-2).

Host-side prep per core: x[b] is transposed (xT [D,T]) and cast to bf16 so the
kernel needs no on-device transposes; weights cast to bf16.

Per-core kernel (Bass/Tile):
  B: Q^T, K^T [I,T] bf16 (heads pair-interleaved per 128-row chunk), V stored
     per-head [128,h,65] bf16 with a ones column for the softmax denominator.
     Emitted per 512-wide t-block, interleaved with attention q-blocks.
  C: per head pair p (rows 0-63 / 64-127 of chunk p) and q-block of 512:
     S^T[k,q] for both heads back-to-back (row-tiled halves of the PE array,
     fp32 PSUM, one bank per head), one exp per k-tile covering both heads
     (scale folded), lower-tri mask multiply on diagonal tiles, then
     ctx^T[65,q] accumulates V_aug^T P^T in PSUM; row 64 is the denominator.
     Normalize via DVE reciprocal + gpsimd partition_broadcast + DVE mul.
  D: out = ctx^T.T Wo accumulated over inner chunks, bf16 partials to DRAM
     (the host sums the two head-group partials in fp32). Projection tiles
     are deferred and used as PE filler during the exp-bound last q-block.

The emission interleaves stages so the in-order PE always has independent
matmul work queued while ACT (exp) is the pacing engine: B(qb+1) fills
attention(qb), the deferred projections fill attention(3).
"""

import numpy as np
import ml_dtypes

import concourse.mybir as mybir
import concourse.tile as tile
from concourse import bacc
from concourse.bass_utils import run_bass_kernel_spmd

B, T, D = 4, 2048, 1024
H_TOT, DH = 16, 64
N_CORES = 8
HPC = 8                  # heads per core
NPAIR = HPC // 2         # head pairs per core (= 128-row chunks of I)
I = HPC * DH             # 512: inner width per core
F32 = mybir.dt.float32
BF16 = mybir.dt.bfloat16
SCALE = float(DH) ** -0.5
QB = 512                 # q-block width
NQB = T // QB            # 4 q-blocks
NTT = T // 128           # 16 t-tiles

_NC_CACHE = []


def _emit(nc, tc, ctx):
    xT_d = nc.dram_tensor("xt", [D, T], BF16, kind="ExternalInput")
    wq_d = nc.dram_tensor("wq", [D, I], BF16, kind="ExternalInput")
    wk_d = nc.dram_tensor("wk", [D, I], BF16, kind="ExternalInput")
    wv_d = nc.dram_tensor("wv", [D, I], BF16, kind="ExternalInput")
    wo_d = nc.dram_tensor("wo", [I, D], BF16, kind="ExternalInput")
    o_d = nc.dram_tensor("o", [T, D], BF16, kind="ExternalOutput")

    xT_view = xT_d.ap().rearrange("(c p) t -> p c t", p=128)   # [128,8,2048]
    o_view = o_d.ap().rearrange("(n p) d -> n p d", p=128)     # [16,128,1024]

    persist = ctx.enter_context(tc.tile_pool(name="persist", bufs=1))

    # tiny constant for the PE-warmup matmuls, built first so they can
    # start immediately
    warmc = persist.tile([128, 64], F32, tag="warmc")
    nc.gpsimd.memset(warmc[:], 1.0)

    # constant: lower-tri keep mask (bf16)
    ltri32 = persist.tile([128, 128], F32, tag="ltri32")
    nc.gpsimd.memset(ltri32[:], 1.0)
    nc.gpsimd.affine_select(
        out=ltri32[:], in_=ltri32[:], compare_op=mybir.AluOpType.is_ge,
        fill=0.0, base=0, pattern=[[1, 128]], channel_multiplier=-1,
    )
    ltri = persist.tile([128, 2, 128], BF16, tag="ltri")
    nc.vector.tensor_copy(ltri[:, 0, :], ltri32[:])
    nc.vector.tensor_copy(ltri[:, 1, :], ltri32[:])
    onescol32 = persist.tile([128, HPC, 1], F32, tag="onescol32")
    nc.gpsimd.memset(onescol32[:], 1.0)
    # touch Exp once so the ACT table set loads during the input DMAs
    actwarm = persist.tile([1, 1], F32, tag="actwarm")
    nc.scalar.activation(actwarm[:], onescol32[0:1, 0, :],
                         mybir.ActivationFunctionType.Exp, scale=1.0)

    # persistent SBUF tensors
    xT = persist.tile([128, 8, T], BF16, tag="xT")              # [D-chunk, T]
    wq_t = persist.tile([128, 8, I], BF16, tag="wq")
    wk_t = persist.tile([128, 8, I], BF16, tag="wk")
    wv_t = persist.tile([128, 8, I], BF16, tag="wv")
    wo_t = persist.tile([128, 4, D], BF16, tag="wo")
    qT = persist.tile([128, NPAIR, T], BF16, tag="qT")          # I-chunk major
    kT = persist.tile([128, NPAIR, T], BF16, tag="kT")
    v3 = persist.tile([128, NTT, HPC, DH + 1], BF16, tag="v3")
    ctxT = persist.tile([128, NPAIR, T], BF16, tag="ctxT")

    # ic0 slices of Wq/Wk arrive as separate host-packed p-major tensors so
    # their DMAs are fully contiguous (a [*, 0:128] slice of wq would pay the
    # <512B-line descriptor penalty); wq_t/wk_t cols 0:128 stay unused.
    wq0_d = nc.dram_tensor("wq0", [128, 8 * 128], BF16, kind="ExternalInput")
    wk0_d = nc.dram_tensor("wk0", [128, 8 * 128], BF16, kind="ExternalInput")
    wq0_t = persist.tile([128, 8, 128], BF16, tag="wq0")
    wk0_t = persist.tile([128, 8, 128], BF16, tag="wk0")

    # weight + x loads, ordered to minimize time-to-first-matmul: the first
    # eighth of x-quarter0 and the packed wq ic0-slice come first.
    def load_x_quarter(tb):
        nc.sync.dma_start(xT[:, :, tb * QB:(tb + 1) * QB],
                          xT_view[:, :, tb * QB:(tb + 1) * QB])
    wq_view = wq_d.ap().rearrange("(c p) i -> p c i", p=128)
    nc.sync.dma_start(xT[:, :, 0:128], xT_view[:, :, 0:128])
    nc.sync.dma_start(wq0_t[:], wq0_d.ap().rearrange("p (c i) -> p c i", c=8))
    nc.sync.dma_start(xT[:, :, 128:256], xT_view[:, :, 128:256])
    nc.sync.dma_start(wk0_t[:], wk0_d.ap().rearrange("p (c i) -> p c i", c=8))
    nc.sync.dma_start(xT[:, :, 256:QB], xT_view[:, :, 256:QB])
    wk_view = wk_d.ap().rearrange("(c p) i -> p c i", p=128)
    nc.sync.dma_start(wq_t[:, :, 128:I], wq_view[:, :, 128:I])
    nc.sync.dma_start(wk_t[:, :, 128:I], wk_view[:, :, 128:I])
    nc.sync.dma_start(wv_t[:], wv_d.ap().rearrange("(c p) i -> p c i", p=128))
    load_x_quarter(1)
    nc.sync.dma_start(wo_t[:], wo_d.ap().rearrange("(c p) d -> p c d", p=128))
    load_x_quarter(2)
    load_x_quarter(3)

    # PSUM bank budget (8): qkv/proj 2 + scores 2x2 + ctx 2 = 8
    psum_qkv = ctx.enter_context(tc.tile_pool(name="psum_qkv", bufs=2, space="PSUM"))
    psum_sc = ctx.enter_context(tc.tile_pool(name="psum_sc", bufs=2, space="PSUM"))
    psum_ctx = ctx.enter_context(tc.tile_pool(name="psum_ctx", bufs=2, space="PSUM"))

    # dummy matmuls on a constant: keep the PE busy during the initial DMA
    # wait so the clock (HAM) is warm when the first real chain starts
    pewarm = psum_qkv.tile([1, 64], F32, tag="qkv", name="pewarm")
    for _ in range(11):
        nc.tensor.matmul(pewarm[:], warmc[:, 0:1], warmc[:],
                         start=True, stop=True)

    ptpool = ctx.enter_context(tc.tile_pool(name="pt", bufs=4))
    recpool = ctx.enter_context(tc.tile_pool(name="rec", bufs=4))
    bcspool = ctx.enter_context(tc.tile_pool(name="bcs", bufs=4))
    outpool = ctx.enter_context(tc.tile_pool(name="out_sb", bufs=3))

    def stage_b_units(tb):
        """QKV for t-block tb (512 wide); yields after each schedulable unit."""
        t0 = tb * QB
        # Q^T / K^T chunks: out rows = I-chunk ic (head pair ic), cols = t
        projs = [(wq_t, wq0_t, qT), (wk_t, wk0_t, kT)]
        order = [(ic, pr) for ic in range(NPAIR) for pr in projs]
        for i, (ic, (w_t, w0_t, dstT)) in enumerate(order):
            # the first two chains run in slices (into one PSUM tile) so they
            # can start as soon as the matching part of x-quarter0 lands
            halves = (((0, 128), (128, 256), (256, QB)) if tb == 0 and i < 2
                      else ((0, QB),))
            ps = psum_qkv.tile([128, QB], F32, tag="qkv", name=f"ps_{tb}_{i}")
            for f0, f1 in halves:
                for dc in range(8):
                    lhsT = (w0_t[:, dc, :] if ic == 0
                            else w_t[:, dc, ic * 128:(ic + 1) * 128])
                    nc.tensor.matmul(
                        ps[:, f0:f1],
                        lhsT,
                        xT[:, dc, t0 + f0:t0 + f1],
                        start=(dc == 0), stop=(dc == 7),
                    )
            cp = nc.vector if tb == 0 else nc.any
            with nc.allow_low_precision(reason="bf16 storage of Q/K"):
                cp.tensor_copy(dstT[:, ic, t0:t0 + QB], ps[:])
            if i % 2 == 1:
                yield
        # V natural per t-tile, per-head columns + ones column
        for tt in range(4 * tb, 4 * tb + 4):
            ps = psum_qkv.tile([128, I], F32, tag="qkv")
            for dc in range(8):
                nc.tensor.matmul(
                    ps[:],
                    xT[:, dc, tt * 128:(tt + 1) * 128],
                    wv_t[:, dc, :],
                    start=(dc == 0), stop=(dc == 7),
                )
            with nc.allow_low_precision(reason="bf16 storage of V"):
                nc.any.tensor_copy(
                    v3[:, tt, :, 0:DH],
                    ps[:].rearrange("p (h d) -> p h d", h=HPC),
                )
            nc.vector.tensor_copy(v3[:, tt, :, DH:DH + 1], onescol32[:])
            yield

    def attention_units(qb, tail_filler=None):
        """All head pairs for q-block qb; yields after each pair. Near the end
        of the last pair, drains `tail_filler` units between k-chunks so the
        in-order PE has independent work queued while exp(ACT) catches up."""
        q0 = qb * QB
        n_kt = 4 * (qb + 1)
        for p in range(NPAIR):
            cps = [psum_ctx.tile([DH + 1, QB], F32, tag="ctx", name=f"cps{qb}_{p}_{i}")
                   for i in range(2)]
            def emit_pv(kt, c0, pt):
                for hl in range(2):
                    nc.tensor.matmul(
                        cps[hl][:, c0:QB], v3[:, kt, 2 * p + hl, :],
                        pt[:, hl, c0:QB],
                        start=(kt == 0), stop=(kt == n_kt - 1),
                    )

            # one-chunk software pipeline: scores(kt+1) is emitted before
            # PV(kt) so the in-order PE has matmul work under each exp(ACT)
            prev_pv = None
            for kt in range(n_kt):
                k0 = kt * 128
                m = kt - 4 * qb  # >= 0: this k-tile touches the diagonal
                c0 = max(m, 0) * 128
                # scores for both heads, row-tiled halves of the PE array
                sc = psum_sc.tile([128, 2, QB], F32, tag="sc")
                for hl in range(2):
                    po = hl * 64
                    nc.tensor.matmul(
                        sc[:, hl, c0:QB],
                        kT[po:po + 64, p, k0:k0 + 128],
                        qT[po:po + 64, p, q0 + c0:q0 + QB],
                        start=True, stop=True,
                    )
                pt = ptpool.tile([128, 2, QB], BF16, tag="pt")
                nc.scalar.activation(
                    pt[:, :, c0:QB], sc[:, :, c0:QB],
                    mybir.ActivationFunctionType.Exp, scale=SCALE,
                )
                if m >= 0:
                    nc.vector.tensor_mul(
                        pt[:, :, c0:c0 + 128],
                        pt[:, :, c0:c0 + 128],
                        ltri[:],
                    )
                if prev_pv is not None:
                    emit_pv(*prev_pv)
                prev_pv = (kt, c0, pt)
                if (tail_filler is not None and p == NPAIR - 1
                        and kt == n_kt - 3):
                    drain(tail_filler, 1)
            emit_pv(*prev_pv)
            # the very last pair gates the trailing output projection; its
            # normalize runs in 256-wide chunks so the first projection tiles
            # (which only need their own q-slice of ctxT) unblock earlier
            last = qb == NQB - 1 and p == NPAIR - 1
            chunks = ((0, 256), (256, QB)) if last else ((0, QB),)
            for n0, n1 in chunks:
                for hl in range(2):
                    po = hl * 64
                    rec = recpool.tile([1, QB], F32, tag="rec",
                                       name=f"rec{qb}_{p}_{hl}_{n0}")
                    nc.vector.reciprocal(rec[:, 0:n1 - n0],
                                         cps[hl][DH:DH + 1, n0:n1])
                    bcs = bcspool.tile([64, QB], F32, tag="bcs",
                                       name=f"bcs{qb}_{p}_{hl}_{n0}")
                    nc.gpsimd.partition_broadcast(bcs[:, 0:n1 - n0],
                                                  rec[:, 0:n1 - n0])
                    with nc.allow_low_precision(reason="bf16 storage of ctx"):
                        nc.vector.tensor_mul(
                            ctxT[po:po + 64, p, q0 + n0:q0 + n1],
                            cps[hl][0:DH, n0:n1], bcs[:, 0:n1 - n0])
            yield

    def stage_d_units(qb):
        """Output projection for the t-tiles of q-block qb; yields per tile."""
        for tt in range(4 * qb, 4 * qb + 4):
            osb = outpool.tile([128, D], BF16, tag="osb")
            for db in range(2):
                ops = psum_qkv.tile([128, 512], F32, tag="qkv")
                for ic in range(NPAIR):
                    nc.tensor.matmul(
                        ops[:],
                        ctxT[:, ic, tt * 128:(tt + 1) * 128],
                        wo_t[:, ic, db * 512:(db + 1) * 512],
                        start=(ic == 0), stop=(ic == NPAIR - 1),
                    )
                with nc.allow_low_precision(reason="bf16 partial output; host sums in fp32"):
                    nc.vector.tensor_copy(osb[:, db * 512:(db + 1) * 512], ops[:])
                if qb == NQB - 1:
                    # tail q-block: ship each half as soon as it is evacuated
                    nc.sync.dma_start(o_view[tt][:, db * 512:(db + 1) * 512],
                                      osb[:, db * 512:(db + 1) * 512])
            if qb != NQB - 1:
                nc.sync.dma_start(o_view[tt], osb[:])
            yield

    def drain(gen, n=None):
        taken = 0
        for _ in gen:
            taken += 1
            if n is not None and taken >= n:
                break

    # Emission: B0 fully, then per q-block interleave attention pairs with
    # filler matmul work for the PE during exp(ACT)-bound stretches. The
    # attention phases grow with qb (causal), so the output projections
    # (D0-D2) are deferred to pad the heaviest phase (qb3); D3 trails.
    drain(stage_b_units(0))
    for qb in range(NQB - 1):
        b_next = stage_b_units(qb + 1)
        for _ in attention_units(qb, tail_filler=b_next):
            drain(b_next, 2)
        drain(b_next)
    d_fill = (u for q in range(NQB - 1) for u in stage_d_units(q))
    for i, _ in enumerate(attention_units(NQB - 1, tail_filler=d_fill)):
        if i < NPAIR - 1:
            drain(d_fill, 2)
    drain(d_fill)
    drain(stage_d_units(NQB - 1))


def _build():
    from contextlib import ExitStack

    nc = bacc.Bacc("TRN2", target_bir_lowering=False, debug=False,
                   enable_asserts=True, num_devices=N_CORES)
    with tile.TileContext(nc) as tc:
        with ExitStack() as ctx:
            _emit(nc, tc, ctx)
    nc.compile()
    return nc


def _get_nc():
    if not _NC_CACHE:
        _NC_CACHE.append(_build())
    return _NC_CACHE[0]


def _in_maps(x, W_q, W_k, W_v, W_o):
    bf = ml_dtypes.bfloat16

    def pmajor_ic0(w):
        # [D, 128] column slice -> [128 p, 8 c, 128 i] contiguous -> [128, 1024]
        return np.ascontiguousarray(
            w.reshape(8, 128, 128).transpose(1, 0, 2)).reshape(128, 1024)

    maps = []
    xts = [np.ascontiguousarray(x[b].T).astype(bf) for b in range(B)]
    for c in range(N_CORES):
        b, g = c // 2, c % 2
        s = slice(g * I, (g + 1) * I)
        wq = np.ascontiguousarray(W_q[:, s]).astype(bf)
        wk = np.ascontiguousarray(W_k[:, s]).astype(bf)
        maps.append({
            "xt": xts[b],
            "wq": wq,
            "wk": wk,
            "wq0": pmajor_ic0(wq[:, 0:128]),
            "wk0": pmajor_ic0(wk[:, 0:128]),
            "wv": np.ascontiguousarray(W_v[:, s]).astype(bf),
            "wo": np.ascontiguousarray(W_o[s, :]).astype(bf),
        })
    return maps


def kernel(**inputs):
    x = np.asarray(inputs["x"], dtype=np.float32)
    W_q = np.asarray(inputs["W_q"], dtype=np.float32)
    W_k = np.asarray(inputs["W_k"], dtype=np.float32)
    W_v = np.asarray(inputs["W_v"], dtype=np.float32)
    W_o = np.asarray(inputs["W_o"], dtype=np.float32)

    nc = _get_nc()
    res = run_bass_kernel_spmd(nc, _in_maps(x, W_q, W_k, W_v, W_o),
                               core_ids=list(range(N_CORES)))
    out = np.empty((B, T, D), dtype=np.float32)
    for b in range(B):
        out[b] = (res.results[2 * b]["o"].astype(np.float32)
                  + res.results[2 * b + 1]["o"].astype(np.float32))
    return out


# revision 64
# speedup vs baseline: 7.3943x; 7.3943x over previous
"""Causal MHA (B=4, T=2048, D=1024, H=16, Dh=64) on 8 TRN2 NeuronCores.

Sharding: tensor-parallel over heads (2 groups of 8 heads; W_q/W_k/W_v split
column-wise, W_o row-wise) x data-parallel over batch (4 batches). Core
c = (b, g) computes a partial output x[b] attention with head-group g; the
host sums the two head-group partials per batch.

All device matmuls run in bf16 (fp32 PSUM accumulation); verified numerically
at ~4e-3 rel err vs the fp32 reference (tolerance 2e-2).

Host-side prep per core: x[b] is transposed (xT [D,T]) and cast to bf16 so the
kernel needs no on-device transposes; weights cast to bf16.

Per-core kernel (Bass/Tile):
  B: Q^T, K^T [I,T] bf16 (heads pair-interleaved per 128-row chunk), V stored
     per-head [128,h,65] bf16 with a ones column for the softmax denominator.
     Emitted per 512-wide t-block, interleaved with attention q-blocks.
  C: per head pair p (rows 0-63 / 64-127 of chunk p) and q-block of 512:
     S^T[k,q] for both heads back-to-back (row-tiled halves of the PE array,
     fp32 PSUM, one bank per head), one exp per k-tile covering both heads
     (scale folded), lower-tri mask multiply on diagonal tiles, then
     ctx^T[65,q] accumulates V_aug^T P^T in PSUM; row 64 is the denominator.
     Normalize via DVE reciprocal + gpsimd partition_broadcast + DVE mul.
  D: out = ctx^T.T Wo accumulated over inner chunks, bf16 partials to DRAM
     (the host sums the two head-group partials in fp32). Projection tiles
     are deferred and used as PE filler during the exp-bound last q-block.

The emission interleaves stages so the in-order PE always has independent
matmul work queued while ACT (exp) is the pacing engine: B(qb+1) fills
attention(qb), the deferred projections fill attention(3).
"""

import numpy as np
import ml_dtypes

import concourse.mybir as mybir
import concourse.tile as tile
from concourse import bacc
from concourse.bass_utils import run_bass_kernel_spmd

B, T, D = 4, 2048, 1024
H_TOT, DH = 16, 64
N_CORES = 8
HPC = 8                  # heads per core
NPAIR = HPC // 2         # head pairs per core (= 128-row chunks of I)
I = HPC * DH             # 512: inner width per core
F32 = mybir.dt.float32
BF16 = mybir.dt.bfloat16
SCALE = float(DH) ** -0.5
QB = 512                 # q-block width
NQB = T // QB            # 4 q-blocks
NTT = T // 128           # 16 t-tiles

_NC_CACHE = []


def _emit(nc, tc, ctx):
    xT_d = nc.dram_tensor("xt", [D, T], BF16, kind="ExternalInput")
    wq_d = nc.dram_tensor("wq", [D, I], BF16, kind="ExternalInput")
    wk_d = nc.dram_tensor("wk", [D, I], BF16, kind="ExternalInput")
    wv_d = nc.dram_tensor("wv", [D, I], BF16, kind="ExternalInput")
    wo_d = nc.dram_tensor("wo", [I, D], BF16, kind="ExternalInput")
    o_d = nc.dram_tensor("o", [T, D], BF16, kind="ExternalOutput")

    xT_view = xT_d.ap().rearrange("(c p) t -> p c t", p=128)   # [128,8,2048]
    o_view = o_d.ap().rearrange("(n p) d -> n p d", p=128)     # [16,128,1024]

    persist = ctx.enter_context(tc.tile_pool(name="persist", bufs=1))

    # tiny constant for the PE-warmup matmuls, built first so they can
    # start immediately
    warmc = persist.tile([128, 64], F32, tag="warmc")
    nc.gpsimd.memset(warmc[:], 1.0)

    # constant: lower-tri keep mask (bf16)
    ltri32 = persist.tile([128, 128], F32, tag="ltri32")
    nc.gpsimd.memset(ltri32[:], 1.0)
    nc.gpsimd.affine_select(
        out=ltri32[:], in_=ltri32[:], compare_op=mybir.AluOpType.is_ge,
        fill=0.0, base=0, pattern=[[1, 128]], channel_multiplier=-1,
    )
    ltri = persist.tile([128, 2, 128], BF16, tag="ltri")
    nc.vector.tensor_copy(ltri[:, 0, :], ltri32[:])
    nc.vector.tensor_copy(ltri[:, 1, :], ltri32[:])
    onescol32 = persist.tile([128, HPC, 1], F32, tag="onescol32")
    nc.gpsimd.memset(onescol32[:], 1.0)
    # touch Exp once so the ACT table set loads during the input DMAs
    actwarm = persist.tile([1, 1], F32, tag="actwarm")
    nc.scalar.activation(actwarm[:], onescol32[0:1, 0, :],
                         mybir.ActivationFunctionType.Exp, scale=1.0)

    # persistent SBUF tensors
    xT = persist.tile([128, 8, T], BF16, tag="xT")              # [D-chunk, T]
    wq_t = persist.tile([128, 8, I], BF16, tag="wq")
    wk_t = persist.tile([128, 8, I], BF16, tag="wk")
    wv_t = persist.tile([128, 8, I], BF16, tag="wv")
    wo_t = persist.tile([128, 4, D], BF16, tag="wo")
    qT = persist.tile([128, NPAIR, T], BF16, tag="qT")          # I-chunk major
    kT = persist.tile([128, NPAIR, T], BF16, tag="kT")
    v3 = persist.tile([128, NTT, HPC, DH + 1], BF16, tag="v3")
    ctxT = persist.tile([128, NPAIR, T], BF16, tag="ctxT")

    # ic0 slices of Wq/Wk arrive as separate host-packed p-major tensors so
    # their DMAs are fully contiguous (a [*, 0:128] slice of wq would pay the
    # <512B-line descriptor penalty); wq_t/wk_t cols 0:128 stay unused.
    wq0_d = nc.dram_tensor("wq0", [128, 8 * 128], BF16, kind="ExternalInput")
    wk0_d = nc.dram_tensor("wk0", [128, 8 * 128], BF16, kind="ExternalInput")
    wq0_t = persist.tile([128, 8, 128], BF16, tag="wq0")
    wk0_t = persist.tile([128, 8, 128], BF16, tag="wk0")

    # weight + x loads, ordered to minimize time-to-first-matmul: the first
    # eighth of x-quarter0 and the packed wq ic0-slice come first.
    def load_x_quarter(tb):
        nc.sync.dma_start(xT[:, :, tb * QB:(tb + 1) * QB],
                          xT_view[:, :, tb * QB:(tb + 1) * QB])
    wq_view = wq_d.ap().rearrange("(c p) i -> p c i", p=128)
    nc.sync.dma_start(xT[:, :, 0:128], xT_view[:, :, 0:128])
    nc.sync.dma_start(wq0_t[:], wq0_d.ap().rearrange("p (c i) -> p c i", c=8))
    nc.sync.dma_start(xT[:, :, 128:256], xT_view[:, :, 128:256])
    nc.sync.dma_start(wk0_t[:], wk0_d.ap().rearrange("p (c i) -> p c i", c=8))
    nc.sync.dma_start(xT[:, :, 256:QB], xT_view[:, :, 256:QB])
    wk_view = wk_d.ap().rearrange("(c p) i -> p c i", p=128)
    nc.sync.dma_start(wq_t[:, :, 128:I], wq_view[:, :, 128:I])
    nc.sync.dma_start(wk_t[:, :, 128:I], wk_view[:, :, 128:I])
    nc.sync.dma_start(wv_t[:], wv_d.ap().rearrange("(c p) i -> p c i", p=128))
    load_x_quarter(1)
    nc.sync.dma_start(wo_t[:], wo_d.ap().rearrange("(c p) d -> p c d", p=128))
    load_x_quarter(2)
    load_x_quarter(3)

    # PSUM bank budget (8): qkv/proj 2 + scores 2x2 + ctx 2 = 8
    psum_qkv = ctx.enter_context(tc.tile_pool(name="psum_qkv", bufs=2, space="PSUM"))
    psum_sc = ctx.enter_context(tc.tile_pool(name="psum_sc", bufs=2, space="PSUM"))
    psum_ctx = ctx.enter_context(tc.tile_pool(name="psum_ctx", bufs=2, space="PSUM"))

    # dummy matmuls on a constant: keep the PE busy during the initial DMA
    # wait so the clock (HAM) is warm when the first real chain starts
    pewarm = psum_qkv.tile([1, 64], F32, tag="qkv", name="pewarm")
    for _ in range(11):
        nc.tensor.matmul(pewarm[:], warmc[:, 0:1], warmc[:],
                         start=True, stop=True)

    ptpool = ctx.enter_context(tc.tile_pool(name="pt", bufs=4))
    recpool = ctx.enter_context(tc.tile_pool(name="rec", bufs=4))
    bcspool = ctx.enter_context(tc.tile_pool(name="bcs", bufs=4))
    outpool = ctx.enter_context(tc.tile_pool(name="out_sb", bufs=3))

    def stage_b_units(tb):
        """QKV for t-block tb (512 wide); yields after each schedulable unit."""
        t0 = tb * QB
        # Q^T / K^T chunks: out rows = I-chunk ic (head pair ic), cols = t
        projs = [(wq_t, wq0_t, qT), (wk_t, wk0_t, kT)]
        order = [(ic, pr) for ic in range(NPAIR) for pr in projs]
        for i, (ic, (w_t, w0_t, dstT)) in enumerate(order):
            # the first two chains run in slices (into one PSUM tile) so they
            # can start as soon as the matching part of x-quarter0 lands
            halves = (((0, 128), (128, 256), (256, QB)) if tb == 0 and i < 2
                      else ((0, QB),))
            ps = psum_qkv.tile([128, QB], F32, tag="qkv", name=f"ps_{tb}_{i}")
            for f0, f1 in halves:
                for dc in range(8):
                    lhsT = (w0_t[:, dc, :] if ic == 0
                            else w_t[:, dc, ic * 128:(ic + 1) * 128])
                    nc.tensor.matmul(
                        ps[:, f0:f1],
                        lhsT,
                        xT[:, dc, t0 + f0:t0 + f1],
                        start=(dc == 0), stop=(dc == 7),
                    )
            cp = nc.vector if tb == 0 else nc.any
            with nc.allow_low_precision(reason="bf16 storage of Q/K"):
                cp.tensor_copy(dstT[:, ic, t0:t0 + QB], ps[:])
            if i % 2 == 1:
                yield
        # V natural per t-tile, per-head columns + ones column
        for tt in range(4 * tb, 4 * tb + 4):
            ps = psum_qkv.tile([128, I], F32, tag="qkv")
            for dc in range(8):
                nc.tensor.matmul(
                    ps[:],
                    xT[:, dc, tt * 128:(tt + 1) * 128],
                    wv_t[:, dc, :],
                    start=(dc == 0), stop=(dc == 7),
                )
            with nc.allow_low_precision(reason="bf16 storage of V"):
                nc.any.tensor_copy(
                    v3[:, tt, :, 0:DH],
                    ps[:].rearrange("p (h d) -> p h d", h=HPC),
                )
            nc.vector.tensor_copy(v3[:, tt, :, DH:DH + 1], onescol32[:])
            yield

    def attention_units(qb, tail_filler=None):
        """All head pairs for q-block qb; yields after each pair. Near the end
        of the last pair, drains `tail_filler` units between k-chunks so the
        in-order PE has independent work queued while exp(ACT) catches up."""
        q0 = qb * QB
        n_kt = 4 * (qb + 1)
        for p in range(NPAIR):
            cps = [psum_ctx.tile([DH + 1, QB], F32, tag="ctx", name=f"cps{qb}_{p}_{i}")
                   for i in range(2)]
            def emit_pv(kt, c0, pt):
                for hl in range(2):
                    nc.tensor.matmul(
                        cps[hl][:, c0:QB], v3[:, kt, 2 * p + hl, :],
                        pt[:, hl, c0:QB],
                        start=(kt == 0), stop=(kt == n_kt - 1),
                    )

            # one-chunk software pipeline: scores(kt+1) is emitted before
            # PV(kt) so the in-order PE has matmul work under each exp(ACT)
            prev_pv = None
            for kt in range(n_kt):
                k0 = kt * 128
                m = kt - 4 * qb  # >= 0: this k-tile touches the diagonal
                c0 = max(m, 0) * 128
                # scores for both heads, row-tiled halves of the PE array
                sc = psum_sc.tile([128, 2, QB], F32, tag="sc")
                for hl in range(2):
                    po = hl * 64
                    nc.tensor.matmul(
                        sc[:, hl, c0:QB],
                        kT[po:po + 64, p, k0:k0 + 128],
                        qT[po:po + 64, p, q0 + c0:q0 + QB],
                        start=True, stop=True,
                    )
                pt = ptpool.tile([128, 2, QB], BF16, tag="pt")
                nc.scalar.activation(
                    pt[:, :, c0:QB], sc[:, :, c0:QB],
                    mybir.ActivationFunctionType.Exp, scale=SCALE,
                )
                if m >= 0:
                    nc.vector.tensor_mul(
                        pt[:, :, c0:c0 + 128],
                        pt[:, :, c0:c0 + 128],
                        ltri[:],
                    )
                if prev_pv is not None:
                    emit_pv(*prev_pv)
                prev_pv = (kt, c0, pt)
                if (tail_filler is not None and p == NPAIR - 1
                        and kt == n_kt - 3):
                    drain(tail_filler, 1)
            emit_pv(*prev_pv)
            # the very last pair gates the trailing output projection; its
            # normalize runs in 256-wide chunks so the first projection tiles
            # (which only need their own q-slice of ctxT) unblock earlier
            last = qb == NQB - 1 and p == NPAIR - 1
            chunks = ((0, 256), (256, QB)) if last else ((0, QB),)
            for n0, n1 in chunks:
                for hl in range(2):
                    po = hl * 64
                    rec = recpool.tile([1, QB], F32, tag="rec",
                                       name=f"rec{qb}_{p}_{hl}_{n0}")
                    nc.vector.reciprocal(rec[:, 0:n1 - n0],
                                         cps[hl][DH:DH + 1, n0:n1])
                    bcs = bcspool.tile([64, QB], F32, tag="bcs",
                                       name=f"bcs{qb}_{p}_{hl}_{n0}")
                    nc.gpsimd.partition_broadcast(bcs[:, 0:n1 - n0],
                                                  rec[:, 0:n1 - n0])
                    with nc.allow_low_precision(reason="bf16 storage of ctx"):
                        nc.vector.tensor_mul(
                            ctxT[po:po + 64, p, q0 + n0:q0 + n1],
                            cps[hl][0:DH, n0:n1], bcs[:, 0:n1 - n0])
            yield

    def stage_d_units(qb):
        """Output projection for the t-tiles of q-block qb; yields per tile."""
        for tt in range(4 * qb, 4 * qb + 4):
            osb = outpool.tile([128, D], BF16, tag="osb")
            for db in range(2):
                ops = psum_qkv.tile([128, 512], F32, tag="qkv")
                for ic in range(NPAIR):
                    nc.tensor.matmul(
                        ops[:],
                        ctxT[:, ic, tt * 128:(tt + 1) * 128],
                        wo_t[:, ic, db * 512:(db + 1) * 512],
                        start=(ic == 0), stop=(ic == NPAIR - 1),
                    )
                with nc.allow_low_precision(reason="bf16 partial output; host sums in fp32"):
                    nc.vector.tensor_copy(osb[:, db * 512:(db + 1) * 512], ops[:])
                if qb == NQB - 1:
                    # tail q-block: ship each half as soon as it is evacuated
                    nc.sync.dma_start(o_view[tt][:, db * 512:(db + 1) * 512],
                                      osb[:, db * 512:(db + 1) * 512])
            if qb != NQB - 1:
                nc.sync.dma_start(o_view[tt], osb[:])
            yield

    def drain(gen, n=None):
        taken = 0
        for _ in gen:
            taken += 1
            if n is not None and taken >= n:
                break

    # Emission: B0 fully, then per q-block interleave attention pairs with
    # filler matmul work for the PE during exp(ACT)-bound stretches. The
    # attention phases grow with qb (causal), so the output projections
    # (D0-D2) are deferred to pad the heaviest phase (qb3); D3 trails.
    drain(stage_b_units(0))
    for qb in range(NQB - 1):
        b_next = stage_b_units(qb + 1)
        for _ in attention_units(qb, tail_filler=b_next):
            drain(b_next, 2)
        drain(b_next)
    d_fill = (u for q in range(NQB - 1) for u in stage_d_units(q))
    for i, _ in enumerate(attention_units(NQB - 1, tail_filler=d_fill)):
        if i < NPAIR - 1:
            drain(d_fill, 2)
    drain(d_fill)
    drain(stage_d_units(NQB - 1))


def _build():
    from contextlib import ExitStack

    nc = bacc.Bacc("TRN2", target_bir_lowering=False, debug=False,
                   enable_asserts=True, num_devices=N_CORES)
    with tile.TileContext(nc) as tc:
        with ExitStack() as ctx:
            _emit(nc, tc, ctx)
    nc.compile()
    return nc


def _get_nc():
    if not _NC_CACHE:
        _NC_CACHE.append(_build())
    return _NC_CACHE[0]


def _in_maps(x, W_q, W_k, W_v, W_o):
    bf = ml_dtypes.bfloat16

    def pmajor_ic0(w):
        # [D, 128] column slice -> [128 p, 8 c, 128 i] contiguous -> [128, 1024]
        return np.ascontiguousarray(
            w.reshape(8, 128, 128).transpose(1, 0, 2)).reshape(128, 1024)

    maps = []
    xts = [np.ascontiguousarray(x[b].T).astype(bf) for b in range(B)]
    for c in range(N_CORES):
        b, g = c // 2, c % 2
        s = slice(g * I, (g + 1) * I)
        wq = np.ascontiguousarray(W_q[:, s]).astype(bf)
        wk = np.ascontiguousarray(W_k[:, s]).astype(bf)
        maps.append({
            "xt": xts[b],
            "wq": wq,
            "wk": wk,
            "wq0": pmajor_ic0(wq[:, 0:128]),
            "wk0": pmajor_ic0(wk[:, 0:128]),
            "wv": np.ascontiguousarray(W_v[:, s]).astype(bf),
            "wo": np.ascontiguousarray(W_o[s, :]).astype(bf),
        })
    return maps


def kernel(**inputs):
    x = np.asarray(inputs["x"], dtype=np.float32)
    W_q = np.asarray(inputs["W_q"], dtype=np.float32)
    W_k = np.asarray(inputs["W_k"], dtype=np.float32)
    W_v = np.asarray(inputs["W_v"], dtype=np.float32)
    W_o = np.asarray(inputs["W_o"], dtype=np.float32)

    nc = _get_nc()
    res = run_bass_kernel_spmd(nc, _in_maps(x, W_q, W_k, W_v, W_o),
                               core_ids=list(range(N_CORES)))
    out = np.empty((B, T, D), dtype=np.float32)
    for b in range(B):
        out[b] = (res.results[2 * b]["o"].astype(np.float32)
                  + res.results[2 * b + 1]["o"].astype(np.float32))
    return out


# revision 66
# speedup vs baseline: 10.5989x; 1.4334x over previous
"""Causal MHA (B=4, T=2048, D=1024, H=16, Dh=64) on 8 TRN2 NeuronCores.

Sharding: tensor-parallel over heads (2 groups of 8 heads; W_q/W_k/W_v split
column-wise, W_o row-wise) x data-parallel over batch (4 batches). Core
c = (b, g) computes a partial output x[b] attention with head-group g; the
host sums the two head-group partials per batch.

All device matmuls run in bf16 (fp32 PSUM accumulation); verified numerically
at ~4e-3 rel err vs the fp32 reference (tolerance 2e-2).

Host-side prep per core: x[b] is transposed (xT [D,T]) and cast to bf16 so the
kernel needs no on-device transposes; weights cast to bf16.

Per-core kernel (Bass/Tile):
  B: Q^T, K^T [I,T] bf16 (heads pair-interleaved per 128-row chunk), V stored
     per-head [128,h,65] bf16 with a ones column for the softmax denominator.
     Emitted per 512-wide t-block, interleaved with attention q-blocks.
  C: per head pair p (rows 0-63 / 64-127 of chunk p) and q-block of 512:
     S^T[k,q] for both heads back-to-back (row-tiled halves of the PE array,
     fp32 PSUM, one bank per head), one exp per k-tile covering both heads
     (scale folded), lower-tri mask multiply on diagonal tiles, then
     ctx^T[65,q] accumulates V_aug^T P^T in PSUM; row 64 is the denominator.
     Normalize via DVE reciprocal + gpsimd partition_broadcast + DVE mul.
  D: out = ctx^T.T Wo accumulated over inner chunks, bf16 partials to DRAM
     (the host sums the two head-group partials in fp32). Projection tiles
     are deferred and used as PE filler during the exp-bound last q-block.

The emission interleaves stages so the in-order PE always has independent
matmul work queued while ACT (exp) is the pacing engine: B(qb+1) fills
attention(qb), the deferred projections fill attention(3).
"""

import numpy as np
import ml_dtypes

import concourse.mybir as mybir
import concourse.tile as tile
from concourse import bacc
from concourse.bass_utils import run_bass_kernel_spmd

B, T, D = 4, 2048, 1024
H_TOT, DH = 16, 64
N_CORES = 8
HPC = 8                  # heads per core
NPAIR = HPC // 2         # head pairs per core (= 128-row chunks of I)
I = HPC * DH             # 512: inner width per core
F32 = mybir.dt.float32
BF16 = mybir.dt.bfloat16
SCALE = float(DH) ** -0.5
QB = 512                 # q-block width
NQB = T // QB            # 4 q-blocks
NTT = T // 128           # 16 t-tiles

_NC_CACHE = []


def _emit(nc, tc, ctx):
    xT_d = nc.dram_tensor("xt", [D, T], BF16, kind="ExternalInput")
    wq_d = nc.dram_tensor("wq", [D, I], BF16, kind="ExternalInput")
    wk_d = nc.dram_tensor("wk", [D, I], BF16, kind="ExternalInput")
    wv_d = nc.dram_tensor("wv", [D, I], BF16, kind="ExternalInput")
    wo_d = nc.dram_tensor("wo", [I, D], BF16, kind="ExternalInput")
    o_d = nc.dram_tensor("o", [T, D], BF16, kind="ExternalOutput")

    xT_view = xT_d.ap().rearrange("(c p) t -> p c t", p=128)   # [128,8,2048]
    o_view = o_d.ap().rearrange("(n p) d -> n p d", p=128)     # [16,128,1024]

    persist = ctx.enter_context(tc.tile_pool(name="persist", bufs=1))

    # tiny constant for the PE-warmup matmuls, built first so they can
    # start immediately
    warmc = persist.tile([128, 64], F32, tag="warmc")
    nc.gpsimd.memset(warmc[:], 1.0)

    # constant: lower-tri keep mask (bf16)
    ltri32 = persist.tile([128, 128], F32, tag="ltri32")
    nc.gpsimd.memset(ltri32[:], 1.0)
    nc.gpsimd.affine_select(
        out=ltri32[:], in_=ltri32[:], compare_op=mybir.AluOpType.is_ge,
        fill=0.0, base=0, pattern=[[1, 128]], channel_multiplier=-1,
    )
    ltri = persist.tile([128, 2, 128], BF16, tag="ltri")
    nc.vector.tensor_copy(ltri[:, 0, :], ltri32[:])
    nc.vector.tensor_copy(ltri[:, 1, :], ltri32[:])
    onescol32 = persist.tile([128, HPC, 1], F32, tag="onescol32")
    nc.gpsimd.memset(onescol32[:], 1.0)
    # touch Exp once so the ACT table set loads during the input DMAs
    actwarm = persist.tile([1, 1], F32, tag="actwarm")
    nc.scalar.activation(actwarm[:], onescol32[0:1, 0, :],
                         mybir.ActivationFunctionType.Exp, scale=1.0)

    # persistent SBUF tensors
    xT = persist.tile([128, 8, T], BF16, tag="xT")              # [D-chunk, T]
    wq_t = persist.tile([128, 8, I], BF16, tag="wq")
    wk_t = persist.tile([128, 8, I], BF16, tag="wk")
    wv_t = persist.tile([128, 8, I], BF16, tag="wv")
    wo_t = persist.tile([128, 4, D], BF16, tag="wo")
    qT = persist.tile([128, NPAIR, T], BF16, tag="qT")          # I-chunk major
    kT = persist.tile([128, NPAIR, T], BF16, tag="kT")
    v3 = persist.tile([128, NTT, HPC, DH + 1], BF16, tag="v3")
    ctxT = persist.tile([128, NPAIR, T], BF16, tag="ctxT")

    # ic0 slices of Wq/Wk arrive as separate host-packed p-major tensors so
    # their DMAs are fully contiguous (a [*, 0:128] slice of wq would pay the
    # <512B-line descriptor penalty); wq_t/wk_t cols 0:128 stay unused.
    wq0_d = nc.dram_tensor("wq0", [128, 8 * 128], BF16, kind="ExternalInput")
    wk0_d = nc.dram_tensor("wk0", [128, 8 * 128], BF16, kind="ExternalInput")
    wq0_t = persist.tile([128, 8, 128], BF16, tag="wq0")
    wk0_t = persist.tile([128, 8, 128], BF16, tag="wk0")

    # weight + x loads, ordered to minimize time-to-first-matmul: the first
    # eighth of x-quarter0 and the packed wq ic0-slice come first.
    def load_x_quarter(tb):
        nc.sync.dma_start(xT[:, :, tb * QB:(tb + 1) * QB],
                          xT_view[:, :, tb * QB:(tb + 1) * QB])
    wq_view = wq_d.ap().rearrange("(c p) i -> p c i", p=128)
    nc.sync.dma_start(xT[:, :, 0:128], xT_view[:, :, 0:128])
    nc.sync.dma_start(wq0_t[:], wq0_d.ap().rearrange("p (c i) -> p c i", c=8))
    nc.sync.dma_start(xT[:, :, 128:256], xT_view[:, :, 128:256])
    nc.sync.dma_start(wk0_t[:], wk0_d.ap().rearrange("p (c i) -> p c i", c=8))
    nc.sync.dma_start(xT[:, :, 256:QB], xT_view[:, :, 256:QB])
    wk_view = wk_d.ap().rearrange("(c p) i -> p c i", p=128)
    nc.sync.dma_start(wq_t[:, :, 128:I], wq_view[:, :, 128:I])
    nc.sync.dma_start(wk_t[:, :, 128:I], wk_view[:, :, 128:I])
    nc.sync.dma_start(wv_t[:], wv_d.ap().rearrange("(c p) i -> p c i", p=128))
    load_x_quarter(1)
    nc.sync.dma_start(wo_t[:], wo_d.ap().rearrange("(c p) d -> p c d", p=128))
    load_x_quarter(2)
    load_x_quarter(3)

    # PSUM bank budget (8): qkv/proj 2 + scores 2x2 + ctx 2 = 8
    psum_qkv = ctx.enter_context(tc.tile_pool(name="psum_qkv", bufs=2, space="PSUM"))
    psum_sc = ctx.enter_context(tc.tile_pool(name="psum_sc", bufs=2, space="PSUM"))
    psum_ctx = ctx.enter_context(tc.tile_pool(name="psum_ctx", bufs=2, space="PSUM"))

    # dummy matmuls on a constant: keep the PE busy during the initial DMA
    # wait so the clock (HAM) is warm when the first real chain starts
    pewarm = psum_qkv.tile([1, 64], F32, tag="qkv", name="pewarm")

    def pe_fill(n):
        for _ in range(n):
            nc.tensor.matmul(pewarm[:], warmc[:, 0:1], warmc[:],
                             start=True, stop=True)
    pe_fill(11)

    ptpool = ctx.enter_context(tc.tile_pool(name="pt", bufs=4))
    recpool = ctx.enter_context(tc.tile_pool(name="rec", bufs=4))
    bcspool = ctx.enter_context(tc.tile_pool(name="bcs", bufs=4))
    outpool = ctx.enter_context(tc.tile_pool(name="out_sb", bufs=3))

    def stage_b_units(tb):
        """QKV for t-block tb (512 wide); yields after each schedulable unit."""
        t0 = tb * QB
        # Q^T / K^T chunks: out rows = I-chunk ic (head pair ic), cols = t
        projs = [(wq_t, wq0_t, qT), (wk_t, wk0_t, kT)]
        order = [(ic, pr) for ic in range(NPAIR) for pr in projs]
        for i, (ic, (w_t, w0_t, dstT)) in enumerate(order):
            # the first two chains run in slices (into one PSUM tile) so they
            # can start as soon as the matching part of x-quarter0 lands
            halves = (((0, 128), (128, 256), (256, QB)) if tb == 0 and i < 2
                      else ((0, QB),))
            ps = psum_qkv.tile([128, QB], F32, tag="qkv", name=f"ps_{tb}_{i}")
            for f0, f1 in halves:
                for dc in range(8):
                    lhsT = (w0_t[:, dc, :] if ic == 0
                            else w_t[:, dc, ic * 128:(ic + 1) * 128])
                    nc.tensor.matmul(
                        ps[:, f0:f1],
                        lhsT,
                        xT[:, dc, t0 + f0:t0 + f1],
                        start=(dc == 0), stop=(dc == 7),
                    )
            cp = nc.vector if tb == 0 else nc.any
            with nc.allow_low_precision(reason="bf16 storage of Q/K"):
                cp.tensor_copy(dstT[:, ic, t0:t0 + QB], ps[:])
            if i % 2 == 1:
                yield
        # V natural per t-tile, per-head columns + ones column
        for tt in range(4 * tb, 4 * tb + 4):
            ps = psum_qkv.tile([128, I], F32, tag="qkv")
            for dc in range(8):
                nc.tensor.matmul(
                    ps[:],
                    xT[:, dc, tt * 128:(tt + 1) * 128],
                    wv_t[:, dc, :],
                    start=(dc == 0), stop=(dc == 7),
                )
            with nc.allow_low_precision(reason="bf16 storage of V"):
                nc.any.tensor_copy(
                    v3[:, tt, :, 0:DH],
                    ps[:].rearrange("p (h d) -> p h d", h=HPC),
                )
            nc.vector.tensor_copy(v3[:, tt, :, DH:DH + 1], onescol32[:])
            yield

    def attention_units(qb, tail_filler=None):
        """All head pairs for q-block qb; yields after each pair. Near the end
        of the last pair, drains `tail_filler` units between k-chunks so the
        in-order PE has independent work queued while exp(ACT) catches up."""
        q0 = qb * QB
        n_kt = 4 * (qb + 1)
        for p in range(NPAIR):
            cps = [psum_ctx.tile([DH + 1, QB], F32, tag="ctx", name=f"cps{qb}_{p}_{i}")
                   for i in range(2)]
            def emit_pv(kt, c0, pt):
                for hl in range(2):
                    nc.tensor.matmul(
                        cps[hl][:, c0:QB], v3[:, kt, 2 * p + hl, :],
                        pt[:, hl, c0:QB],
                        start=(kt == 0), stop=(kt == n_kt - 1),
                    )

            # one-chunk software pipeline: scores(kt+1) is emitted before
            # PV(kt) so the in-order PE has matmul work under each exp(ACT)
            prev_pv = None
            for kt in range(n_kt):
                k0 = kt * 128
                m = kt - 4 * qb  # >= 0: this k-tile touches the diagonal
                c0 = max(m, 0) * 128
                # scores for both heads, row-tiled halves of the PE array
                sc = psum_sc.tile([128, 2, QB], F32, tag="sc")
                for hl in range(2):
                    po = hl * 64
                    nc.tensor.matmul(
                        sc[:, hl, c0:QB],
                        kT[po:po + 64, p, k0:k0 + 128],
                        qT[po:po + 64, p, q0 + c0:q0 + QB],
                        start=True, stop=True,
                    )
                pt = ptpool.tile([128, 2, QB], BF16, tag="pt")
                nc.scalar.activation(
                    pt[:, :, c0:QB], sc[:, :, c0:QB],
                    mybir.ActivationFunctionType.Exp, scale=SCALE,
                )
                if m >= 0:
                    nc.vector.tensor_mul(
                        pt[:, :, c0:c0 + 128],
                        pt[:, :, c0:c0 + 128],
                        ltri[:],
                    )
                if prev_pv is not None:
                    emit_pv(*prev_pv)
                prev_pv = (kt, c0, pt)
                if (tail_filler is not None and p == NPAIR - 1
                        and kt == n_kt - 3):
                    drain(tail_filler, 1)
            emit_pv(*prev_pv)
            # the very last pair gates the trailing output projection; its
            # normalize runs in 256-wide chunks so the first projection tiles
            # (which only need their own q-slice of ctxT) unblock earlier
            last = qb == NQB - 1 and p == NPAIR - 1
            chunks = ((0, 256), (256, QB)) if last else ((0, QB),)
            for n0, n1 in chunks:
                for hl in range(2):
                    po = hl * 64
                    rec = recpool.tile([1, QB], F32, tag="rec",
                                       name=f"rec{qb}_{p}_{hl}_{n0}")
                    nc.vector.reciprocal(rec[:, 0:n1 - n0],
                                         cps[hl][DH:DH + 1, n0:n1])
                    bcs = bcspool.tile([64, QB], F32, tag="bcs",
                                       name=f"bcs{qb}_{p}_{hl}_{n0}")
                    nc.gpsimd.partition_broadcast(bcs[:, 0:n1 - n0],
                                                  rec[:, 0:n1 - n0])
                    with nc.allow_low_precision(reason="bf16 storage of ctx"):
                        nc.vector.tensor_mul(
                            ctxT[po:po + 64, p, q0 + n0:q0 + n1],
                            cps[hl][0:DH, n0:n1], bcs[:, 0:n1 - n0])
            yield

    def stage_d_units(qb):
        """Output projection for the t-tiles of q-block qb; yields per tile."""
        for tt in range(4 * qb, 4 * qb + 4):
            osb = outpool.tile([128, D], BF16, tag="osb")
            for db in range(2):
                ops = psum_qkv.tile([128, 512], F32, tag="qkv")
                for ic in range(NPAIR):
                    nc.tensor.matmul(
                        ops[:],
                        ctxT[:, ic, tt * 128:(tt + 1) * 128],
                        wo_t[:, ic, db * 512:(db + 1) * 512],
                        start=(ic == 0), stop=(ic == NPAIR - 1),
                    )
                with nc.allow_low_precision(reason="bf16 partial output; host sums in fp32"):
                    nc.vector.tensor_copy(osb[:, db * 512:(db + 1) * 512], ops[:])
                if qb == NQB - 1:
                    # tail q-block: ship each half as soon as it is evacuated
                    nc.sync.dma_start(o_view[tt][:, db * 512:(db + 1) * 512],
                                      osb[:, db * 512:(db + 1) * 512])
            if qb != NQB - 1:
                nc.sync.dma_start(o_view[tt], osb[:])
            yield

    def drain(gen, n=None):
        taken = 0
        for _ in gen:
            taken += 1
            if n is not None and taken >= n:
                break

    # Emission: B0 fully, then per q-block interleave attention pairs with
    # filler matmul work for the PE during exp(ACT)-bound stretches. The
    # attention phases grow with qb (causal), so the output projections
    # (D0-D2) are deferred to pad the heaviest phase (qb3); D3 trails.
    drain(stage_b_units(0))
    for qb in range(NQB - 1):
        b_next = stage_b_units(qb + 1)
        for _ in attention_units(qb, tail_filler=b_next):
            drain(b_next, 2)
        drain(b_next)
    d_fill = (u for q in range(NQB - 1) for u in stage_d_units(q))
    for i, _ in enumerate(attention_units(NQB - 1, tail_filler=d_fill)):
        if i < NPAIR - 1:
            drain(d_fill, 2)
    drain(d_fill)
    drain(stage_d_units(NQB - 1))


def _build():
    from contextlib import ExitStack

    nc = bacc.Bacc("TRN2", target_bir_lowering=False, debug=False,
                   enable_asserts=True, num_devices=N_CORES)
    with tile.TileContext(nc) as tc:
        with ExitStack() as ctx:
            _emit(nc, tc, ctx)
    nc.compile()
    return nc


def _get_nc():
    if not _NC_CACHE:
        _NC_CACHE.append(_build())
    return _NC_CACHE[0]


def _in_maps(x, W_q, W_k, W_v, W_o):
    bf = ml_dtypes.bfloat16

    def pmajor_ic0(w):
        # [D, 128] column slice -> [128 p, 8 c, 128 i] contiguous -> [128, 1024]
        return np.ascontiguousarray(
            w.reshape(8, 128, 128).transpose(1, 0, 2)).reshape(128, 1024)

    maps = []
    xts = [np.ascontiguousarray(x[b].T).astype(bf) for b in range(B)]
    for c in range(N_CORES):
        b, g = c // 2, c % 2
        s = slice(g * I, (g + 1) * I)
        wq = np.ascontiguousarray(W_q[:, s]).astype(bf)
        wk = np.ascontiguousarray(W_k[:, s]).astype(bf)
        maps.append({
            "xt": xts[b],
            "wq": wq,
            "wk": wk,
            "wq0": pmajor_ic0(wq[:, 0:128]),
            "wk0": pmajor_ic0(wk[:, 0:128]),
            "wv": np.ascontiguousarray(W_v[:, s]).astype(bf),
            "wo": np.ascontiguousarray(W_o[s, :]).astype(bf),
        })
    return maps


def kernel(**inputs):
    x = np.asarray(inputs["x"], dtype=np.float32)
    W_q = np.asarray(inputs["W_q"], dtype=np.float32)
    W_k = np.asarray(inputs["W_k"], dtype=np.float32)
    W_v = np.asarray(inputs["W_v"], dtype=np.float32)
    W_o = np.asarray(inputs["W_o"], dtype=np.float32)

    nc = _get_nc()
    res = run_bass_kernel_spmd(nc, _in_maps(x, W_q, W_k, W_v, W_o),
                               core_ids=list(range(N_CORES)))
    out = np.empty((B, T, D), dtype=np.float32)
    for b in range(B):
        out[b] = (res.results[2 * b]["o"].astype(np.float32)
                  + res.results[2 * b + 1]["o"].astype(np.float32))
    return out
